# revision 35
# baseline (speedup 1.0000x reference)
"""Trainium2 Bass kernel for nn_AttentionBlock (sparse attention w/ gaussian bias).

Reference computation (per batch b):
    qp = q @ Wq + bq; kp = k @ Wk + bk; vp = v @ Wv + bv          (d_model=512 -> dk=dv=64)
    attn = qp @ kp^T / 8 + g_bias / (2 tau^2); attn[mask] = -inf
    p = softmax(attn, axis=-1)
    out = (p @ vp) @ Wfc + bfc

Sharding: 8 cores = (batch b in 0..3) x (query-half h in 0..1), fully
independent (K/V replicated per core; no collectives).

Layout strategy: everything is computed TRANSPOSED so no big PE transposes
are needed anywhere:
  - q/k/v are host-cast to bf16 and loaded pre-transposed via DMA-transpose
    (xT[d, rows]); projections contract d directly.
  - scores are built transposed, one sk-tile [128, 1024] at a time:
        sT[128 sk, 1024 sq] = kpT_chunk.T @ qpT  (+ I @ gmT accumulate)
    where gmT = g_bias^T (bf16, host-transposed) - 1e30*mask^T.
  - e = exp(sT/(2 tau^2) - 3) in f16; PV accumulates oT[65, 1024] over all
    16 sk-tiles in one PSUM tile; row 64 (ones-column of V) = softmax
    denominators for free.
  - FC consumes oT directly as lhsT; per-row 1/denominator applied on the
    FC output; denominators extracted with 8 tiny PE transposes.

Scheduling: DMA ring FIFOs and the DVE queue are manually ordered so the
critical chain (kT -> kpT, qT -> qpT, gb0/mask0 -> gm0) clears by ~10us and
the 16 score tiles then pace the kernel; V is projected per-tile inside the
main loop since vT lands after scores begin.
"""
import numpy as np

B, S, D, DKV = 4, 2048, 512, 64
SQL = S // 2          # query rows per core
NT_SK = S // 128      # 16 sk tiles
NG_GB = 4             # gb/mask DMA groups (4 sk-tiles each)
N_CORES = 8


def _build():
    import concourse.bass as bass
    import concourse.mybir as mybir
    import concourse.tile as tile
    from concourse import bacc
    from concourse.masks import make_identity
    from contextlib import ExitStack

    f32, bf16, u8 = mybir.dt.float32, mybir.dt.bfloat16, mybir.dt.uint8
    f16 = mybir.dt.float16
    f32r = mybir.dt.float32r
    AF = mybir.ActivationFunctionType
    OP = mybir.AluOpType

    nc = bacc.Bacc(num_devices=N_CORES)
    # q/k/v arrive host-transposed: [d_model, rows] bf16
    q_ext = nc.declare_dram_parameter("q", [D, SQL], bf16, isOutput=False)
    k_ext = nc.declare_dram_parameter("k", [D, S], bf16, isOutput=False)
    v_ext = nc.declare_dram_parameter("v", [D, S], bf16, isOutput=False)
    gbt_ext = nc.declare_dram_parameter("gbt", [S, SQL], bf16, isOutput=False)
    mt_ext = nc.declare_dram_parameter("maskt", [S, SQL], bf16, isOutput=False)
    # w3 = [Wq | Wk | Wv] stacked on the output dim, bf16
    w3_ext = nc.declare_dram_parameter("w3", [D, 3 * DKV], bf16, isOutput=False)
    wfc_ext = nc.declare_dram_parameter("Wfc", [DKV, D], bf16, isOutput=False)
    # consts = [bq | bk | qscale | escale | bvb | bfcb] packed, f32
    NC_CONST = 4 + DKV + D
    consts_ext = nc.declare_dram_parameter("consts", [128, NC_CONST], f32,
                                           isOutput=False)
    out_ext = nc.declare_dram_parameter("out", [SQL, D], f32, isOutput=True)

    with tile.TileContext(nc) as tc:
        with ExitStack() as ctx:
            wpool = ctx.enter_context(tc.tile_pool(name="weights", bufs=1))
            ppool = ctx.enter_context(tc.tile_pool(name="proj", bufs=1))

            # ---- memsets / identity first (cheap, unblock PE warm-up) ----
            warm_t = wpool.tile([128, 512], bf16, tag="warm")
            nc.gpsimd.memset(warm_t[:], 0.001)
            eb_t = wpool.tile([128, 1], f32, tag="eb")
            nc.gpsimd.memset(eb_t[:], -3.0)
            ident = wpool.tile([128, 128], f32, tag="ident")
            make_identity(nc, ident[:])
            ident_bf = wpool.tile([128, 128], bf16, tag="ident_bf")
            # ident_m = -1e30 * I: adds -1e30*mask into score PSUM via matmul
            ident_m = wpool.tile([128, 128], bf16, tag="ident_m")
            nc.vector.tensor_copy(ident_bf[:], ident[:])
            nc.vector.tensor_scalar(out=ident_m[:], in0=ident[:], scalar1=-1e30,
                                    scalar2=None, op0=OP.mult)

            # ---- packed weights / constants (gpsimd ring) ----
            w3_t = wpool.tile([128, 4, 3 * DKV], bf16, tag="w3")
            nc.gpsimd.dma_start(w3_t[:], w3_ext.rearrange("(c p) n -> p c n", p=128))
            wq_t = w3_t[:, :, 0:DKV]
            wk_t = w3_t[:, :, DKV:2 * DKV]
            wv_t = w3_t[:, :, 2 * DKV:3 * DKV]
            consts_t = wpool.tile([128, NC_CONST], f32, tag="consts")
            nc.gpsimd.dma_start(consts_t[:], consts_ext[:])
            bq_t = consts_t[0:DKV, 0:1]
            bk_t = consts_t[0:DKV, 1:2]
            qs_t = consts_t[0:DKV, 2:3]
            es_t = consts_t[:, 3:4]
            bv_t = consts_t[:, 4:4 + DKV]
            bfc_t = consts_t[:, 4 + DKV:NC_CONST]

            # ---- persistent projected tensors ----
            kpT = ppool.tile([DKV, S], f32r, tag="kpT")        # [64, 2048]
            qpT = ppool.tile([DKV, SQL], f32r, tag="qpT")      # [64, 1024]
            vp_aug = ppool.tile([128, NT_SK, DKV + 1], f16, tag="vp")
            # 66 partitions: even sizes keep matmul/transpose ISA checks happy
            oT_sb = ppool.tile([DKV + 2, SQL], bf16, tag="oT")
            recip_sb = ppool.tile([128, SQL // 128], f32, tag="recip")
            wfc_b = wpool.tile([DKV, D], bf16, tag="wfc_b")

            nc.gpsimd.memset(vp_aug[:, :, DKV:DKV + 1], 1.0)

            with tc.tile_pool(name="pa_kv", bufs=1) as pkv_pool, \
                 tc.tile_pool(name="pa_q", bufs=1) as pq_pool, \
                 tc.tile_pool(name="pb_gb", bufs=3) as pgb, \
                 tc.tile_pool(name="pb_m", bufs=3) as pm, \
                 tc.tile_pool(name="pb_e", bufs=16) as pe_pool, \
                 tc.tile_pool(name="pb_o", bufs=2) as po_pool, \
                 tc.tile_pool(name="ps_s", bufs=2, space="PSUM") as ps_s, \
                 tc.tile_pool(name="ps_pv", bufs=1, space="PSUM") as ps_pv, \
                 tc.tile_pool(name="ps_fc", bufs=2, space="PSUM") as ps_fc:

                # ---- plain loads of host-transposed q/k/v ----
                # (DMA-transpose is unusable here: it serializes against all
                # concurrent DMA traffic, and cross-ring transposes race.)
                kT_h0 = pkv_pool.tile([128, 4, SQL], bf16, tag="kT0")
                kT_h1 = pkv_pool.tile([128, 4, SQL], bf16, tag="kT1")
                kT = [kT_h0, kT_h1]
                nc.sync.dma_start(
                    kT[0][:], k_ext[:, 0:SQL].rearrange("(c p) r -> p c r", p=128))
                qT = pq_pool.tile([128, 4, SQL], bf16, tag="qT")
                nc.scalar.dma_start(
                    qT[:], q_ext.rearrange("(c p) r -> p c r", p=128))

                # gb/mask DMAs: mask arrives bf16 {0,1} from the host so
                # both are plain fast HWDGE loads (SWDGE cast was 20us late,
                # DVE casting was 4us/group).
                gb_tiles, m_tiles = {}, {}

                def issue_group_dma(g, gb_eng, m_eng):
                    rows = slice(512 * g, 512 * (g + 1))
                    m_t = pm.tile([128, 4, SQL], bf16, tag="m")
                    m_eng.dma_start(
                        m_t[:], mt_ext[rows].rearrange("(t p) s -> p t s", p=128))
                    gb_t = pgb.tile([128, 4, SQL], bf16, tag="gb")
                    gb_eng.dma_start(
                        gb_t[:], gbt_ext[rows].rearrange("(t p) s -> p t s", p=128))
                    gb_tiles[g], m_tiles[g] = gb_t, m_t

                issue_group_dma(0, nc.sync, nc.sync)
                nc.sync.dma_start(
                    kT[1][:], k_ext[:, SQL:S].rearrange("(c p) r -> p c r", p=128))
                nc.gpsimd.dma_start(wfc_b[:], wfc_ext[:])

                vT = pkv_pool.tile([128, 4, S], bf16, tag="vT")
                nc.scalar.dma_start(
                    vT[:], v_ext.rearrange("(c p) r -> p c r", p=128))

                # ---- warm-up + k/q projections (PSUM shared with phase B) ----
                if True:
                    # HAM warm-up: ~4us of junk matmuls while input DMAs fly,
                    # so projections + phase B run at 2.4GHz from the start
                    ps_w = ps_fc.tile([128, 512], f32, tag="fc")
                    for i in range(30):
                        nc.tensor.matmul(ps_w[:], warm_t[:, 0:128], warm_t[:],
                                         start=True, stop=True)
                    # kpT half 0 first: unblocks score tiles 0..7
                    def kproj(half):
                        hsl = slice(SQL * half, SQL * (half + 1))
                        pp = ps_s.tile([DKV, SQL], f32, tag="s")
                        for c in range(2):
                            sl = slice(512 * c, 512 * (c + 1))
                            for j in range(4):
                                nc.tensor.matmul(pp[:, sl], wk_t[:, j, :],
                                                 kT[half][:, j, sl],
                                                 start=(j == 0), stop=(j == 3))
                        nc.vector.tensor_scalar(out=kpT[:, hsl], in0=pp[:],
                                                scalar1=bk_t, scalar2=None,
                                                op0=OP.add)

                    kproj(0)

                    pp = ps_s.tile([DKV, SQL], f32, tag="s")
                    for c in range(2):
                        sl = slice(512 * c, 512 * (c + 1))
                        for j in range(4):
                            nc.tensor.matmul(pp[:, sl], wq_t[:, j, :], qT[:, j, sl],
                                             start=(j == 0), stop=(j == 3))
                    nc.vector.tensor_scalar(out=qpT[:], in0=pp[:], scalar1=bq_t,
                                            scalar2=qs_t, op0=OP.add, op1=OP.mult)

                # ---- phase B: scores, exp, per-tile V projection, PV ----
                if True:
                    oT_ps = ps_pv.tile([DKV + 1, SQL], f32, tag="oTp")
                    e_tiles = {}

                    def pv_mm(j):
                        # PV for tile j, issued one iteration late so the PE
                        # never stalls in-FIFO waiting on ACT's exp.
                        for c in range(2):
                            sl = slice(512 * c, 512 * (c + 1))
                            nc.tensor.matmul(oT_ps[:, sl], vp_aug[:, j, :],
                                             e_tiles[j][:, sl], start=(j == 0),
                                             stop=(j == NT_SK - 1))

                    def vp_proj(j):
                        pv = ps_fc.tile([128, DKV], f32, tag="fc")
                        for vj in range(4):
                            nc.tensor.matmul(pv[:],
                                             vT[:, vj, 128 * j:128 * (j + 1)],
                                             wv_t[:, vj, :], start=(vj == 0),
                                             stop=(vj == 3))
                        nc.vector.tensor_tensor(out=vp_aug[:, j, 0:DKV],
                                                in0=pv[:], in1=bv_t, op=OP.add)

                    for g in range(NG_GB):
                        for t in range(4):
                            j = 4 * g + t
                            gb_t, m_t = gb_tiles[g], m_tiles[g]
                            ps = ps_s.tile([128, SQL], f32, tag="s")
                            for c in range(2):
                                sl = slice(512 * c, 512 * (c + 1))
                                nc.tensor.matmul(ps[:, sl],
                                                 kpT[:, 128 * j:128 * (j + 1)],
                                                 qpT[:, sl], start=True, stop=False)
                            for c in range(2):
                                sl = slice(512 * c, 512 * (c + 1))
                                nc.tensor.matmul(ps[:, sl], ident_bf[:],
                                                 gb_t[:, t, sl],
                                                 start=False, stop=False)
                            for c in range(2):
                                sl = slice(512 * c, 512 * (c + 1))
                                nc.tensor.matmul(ps[:, sl], ident_m[:],
                                                 m_t[:, t, sl],
                                                 start=False, stop=True)
                            # V projection, one tile ahead (deferred past
                            # tile0 so its fused PE wait excludes vT)
                            if j == 1:
                                vp_proj(0)
                                vp_proj(1)
                            elif j > 1:
                                vp_proj(j)
                            e_t = pe_pool.tile([128, SQL], f16, tag="e")
                            nc.scalar.activation(e_t[:], ps[:], AF.Exp,
                                                 bias=eb_t[:], scale=es_t)
                            e_tiles[j] = e_t
                            if j > 0:
                                pv_mm(j - 1)
                            # stage upcoming groups (issued inside the loop
                            # so tile0's fused PE wait can't include them)
                            if g == 0 and t == 2:
                                kproj(1)
                            if j in (0, 1) and j + 1 < NG_GB:
                                issue_group_dma(j + 1, nc.scalar, nc.scalar)
                            if j == 4:
                                issue_group_dma(3, nc.scalar, nc.scalar)
                    pv_mm(NT_SK - 1)

                    # ---- tail: denominators, FC, store ----
                    nc.scalar.copy(oT_sb[0:DKV + 1, :], oT_ps[:])
                    for t in range(SQL // 128):
                        csl = slice(128 * t, 128 * (t + 1))
                        tr = ps_fc.tile([128, DKV + 2], bf16, tag="fc")
                        nc.tensor.transpose(tr[:], oT_sb[:, csl],
                                            ident_bf[0:DKV + 2, 0:DKV + 2])
                        nc.vector.reciprocal(recip_sb[:, t:t + 1],
                                             tr[:, DKV:DKV + 1])
                    for t in range(SQL // 128):
                        csl = slice(128 * t, 128 * (t + 1))
                        pf = ps_s.tile([128, D], f32, tag="s")
                        nc.tensor.matmul(pf[:], oT_sb[0:DKV, csl], wfc_b[:],
                                         start=True, stop=True)
                        o_sb = po_pool.tile([128, D], f32, tag="o")
                        nc.vector.scalar_tensor_tensor(
                            out=o_sb[:], in0=pf[:], scalar=recip_sb[:, t:t + 1],
                            in1=bfc_t, op0=OP.mult, op1=OP.add)
                        nc.sync.dma_start(out_ext[csl, :], o_sb[:])

    nc.finalize()
    return nc


_cache = {}


def _bf16(x):
    import ml_dtypes
    return np.ascontiguousarray(np.asarray(x, np.float32).astype(ml_dtypes.bfloat16))


def kernel(**inputs):
    from concourse.bass_utils import run_bass_kernel_spmd

    q = np.asarray(inputs["q"], np.float32)
    k = np.asarray(inputs["k"], np.float32)
    v = np.asarray(inputs["v"], np.float32)
    gb = np.asarray(inputs["g_bias"], np.float32)
    mask = np.asarray(inputs["mask"]).astype(np.uint8)
    tau = float(np.asarray(inputs["tau"]))

    if "nc" not in _cache:
        _cache["nc"] = _build()
    nc = _cache["nc"]

    in_maps = build_in_maps(inputs, q, k, v, gb, mask, tau)
    res = run_bass_kernel_spmd(nc, in_maps, list(range(N_CORES)))
    out = np.empty((B, S, D), np.float32)
    for c in range(N_CORES):
        b, h = divmod(c, 2)
        out[b, h * SQL:(h + 1) * SQL] = res.results[c]["out"]
    return out


def build_in_maps(inputs, q, k, v, gb, mask, tau):
    consts = np.zeros((128, 4 + DKV + D), np.float32)
    consts[0:DKV, 0] = np.asarray(inputs["bq"], np.float32)
    consts[0:DKV, 1] = np.asarray(inputs["bk"], np.float32)
    consts[0:DKV, 2] = (2.0 * tau * tau) / 8.0
    consts[:, 3] = 1.0 / (2.0 * tau * tau)
    consts[:, 4:4 + DKV] = np.asarray(inputs["bv"], np.float32)
    consts[:, 4 + DKV:] = np.asarray(inputs["bfc"], np.float32)
    w3 = np.concatenate([np.asarray(inputs["Wq"], np.float32),
                         np.asarray(inputs["Wk"], np.float32),
                         np.asarray(inputs["Wv"], np.float32)], axis=1)
    shared = {
        "w3": _bf16(w3),
        "Wfc": _bf16(inputs["Wfc"]),
        "consts": consts,
    }
    kb = [_bf16(k[b].T) for b in range(B)]
    vb = [_bf16(v[b].T) for b in range(B)]
    in_maps = []
    for c in range(N_CORES):
        b, h = divmod(c, 2)
        sl = slice(h * SQL, (h + 1) * SQL)
        in_maps.append({
            "q": _bf16(q[b, sl].T),
            "k": kb[b],
            "v": vb[b],
            "gbt": _bf16(gb[b, sl].T),
            "maskt": _bf16(mask[b, sl].T),
            **shared,
        })
    return in_maps


# revision 36
# speedup vs baseline: 1.1684x; 1.1684x over previous
"""Trainium2 Bass kernel for nn_AttentionBlock (sparse attention w/ gaussian bias).

Reference computation (per batch b):
    qp = q @ Wq + bq; kp = k @ Wk + bk; vp = v @ Wv + bv          (d_model=512 -> dk=dv=64)
    attn = qp @ kp^T / 8 + g_bias / (2 tau^2); attn[mask] = -inf
    p = softmax(attn, axis=-1)
    out = (p @ vp) @ Wfc + bfc

Sharding: 8 cores = (batch b in 0..3) x (query-half h in 0..1), fully
independent (K/V replicated per core; no collectives).

Layout strategy: everything is computed TRANSPOSED so no big PE transposes
are needed anywhere:
  - q/k/v are host-cast to bf16 and loaded pre-transposed via DMA-transpose
    (xT[d, rows]); projections contract d directly.
  - scores are built transposed, one sk-tile [128, 1024] at a time:
        sT[128 sk, 1024 sq] = kpT_chunk.T @ qpT  (+ I @ gmT accumulate)
    where gmT = g_bias^T (bf16, host-transposed) - 1e30*mask^T.
  - e = exp(sT/(2 tau^2) - 3) in f16; PV accumulates oT[65, 1024] over all
    16 sk-tiles in one PSUM tile; row 64 (ones-column of V) = softmax
    denominators for free.
  - FC consumes oT directly as lhsT; per-row 1/denominator applied on the
    FC output; denominators extracted with 8 tiny PE transposes.

Scheduling: DMA ring FIFOs and the DVE queue are manually ordered so the
critical chain (kT -> kpT, qT -> qpT, gb0/mask0 -> gm0) clears by ~10us and
the 16 score tiles then pace the kernel; V is projected per-tile inside the
main loop since vT lands after scores begin.
"""
import numpy as np

B, S, D, DKV = 4, 2048, 512, 64
SQL = S // 2          # query rows per core
NT_SK = S // 128      # 16 sk tiles
NG_GB = 4             # gb/mask DMA groups (4 sk-tiles each)
N_CORES = 8


def _build():
    import concourse.bass as bass
    import concourse.mybir as mybir
    import concourse.tile as tile
    from concourse import bacc
    from concourse.masks import make_identity
    from contextlib import ExitStack

    f32, bf16, u8 = mybir.dt.float32, mybir.dt.bfloat16, mybir.dt.uint8
    f16 = mybir.dt.float16
    f32r = mybir.dt.float32r
    AF = mybir.ActivationFunctionType
    OP = mybir.AluOpType

    nc = bacc.Bacc(num_devices=N_CORES)
    # q/k/v arrive host-transposed: [d_model, rows] bf16
    q_ext = nc.declare_dram_parameter("q", [D, SQL], bf16, isOutput=False)
    k_ext = nc.declare_dram_parameter("k", [D, S], bf16, isOutput=False)
    v_ext = nc.declare_dram_parameter("v", [D, S], bf16, isOutput=False)
    gbt_ext = nc.declare_dram_parameter("gbt", [S, SQL], bf16, isOutput=False)
    mt_ext = nc.declare_dram_parameter("maskt", [S, SQL], bf16, isOutput=False)
    # w3 = [Wq | Wk | Wv] stacked on the output dim, bf16
    w3_ext = nc.declare_dram_parameter("w3", [D, 3 * DKV], bf16, isOutput=False)
    wfc_ext = nc.declare_dram_parameter("Wfc", [DKV, D], bf16, isOutput=False)
    # consts = [bq | bk | qscale | escale | bvb | bfcb] packed, f32
    NC_CONST = 4 + DKV + D
    consts_ext = nc.declare_dram_parameter("consts", [128, NC_CONST], f32,
                                           isOutput=False)
    out_ext = nc.declare_dram_parameter("out", [SQL, D], f32, isOutput=True)

    with tile.TileContext(nc) as tc:
        with ExitStack() as ctx:
            wpool = ctx.enter_context(tc.tile_pool(name="weights", bufs=1))
            ppool = ctx.enter_context(tc.tile_pool(name="proj", bufs=1))

            # ---- memsets / identity first (cheap, unblock PE warm-up) ----
            warm_t = wpool.tile([128, 512], bf16, tag="warm")
            nc.gpsimd.memset(warm_t[:], 0.001)
            eb_t = wpool.tile([128, 1], f32, tag="eb")
            nc.gpsimd.memset(eb_t[:], -3.0)
            ident = wpool.tile([128, 128], f32, tag="ident")
            make_identity(nc, ident[:])
            ident_bf = wpool.tile([128, 128], bf16, tag="ident_bf")
            # ident_m = -1e30 * I: adds -1e30*mask into score PSUM via matmul
            ident_m = wpool.tile([128, 128], bf16, tag="ident_m")
            nc.vector.tensor_copy(ident_bf[:], ident[:])
            nc.vector.tensor_scalar(out=ident_m[:], in0=ident[:], scalar1=-1e30,
                                    scalar2=None, op0=OP.mult)

            # ---- packed weights / constants (gpsimd ring) ----
            w3_t = wpool.tile([128, 4, 3 * DKV], bf16, tag="w3")
            nc.gpsimd.dma_start(w3_t[:], w3_ext.rearrange("(c p) n -> p c n", p=128))
            wq_t = w3_t[:, :, 0:DKV]
            wk_t = w3_t[:, :, DKV:2 * DKV]
            wv_t = w3_t[:, :, 2 * DKV:3 * DKV]
            consts_t = wpool.tile([128, NC_CONST], f32, tag="consts")
            nc.gpsimd.dma_start(consts_t[:], consts_ext[:])
            bq_t = consts_t[0:DKV, 0:1]
            bk_t = consts_t[0:DKV, 1:2]
            qs_t = consts_t[0:DKV, 2:3]
            es_t = consts_t[:, 3:4]
            bv_t = consts_t[:, 4:4 + DKV]
            bfc_t = consts_t[:, 4 + DKV:NC_CONST]

            # ---- persistent projected tensors ----
            kpT = ppool.tile([DKV, S], f32r, tag="kpT")        # [64, 2048]
            qpT = ppool.tile([DKV, SQL], f32r, tag="qpT")      # [64, 1024]
            vp_aug = ppool.tile([128, NT_SK, DKV + 1], f16, tag="vp")
            # 66 partitions: even sizes keep matmul/transpose ISA checks happy
            oT_sb = ppool.tile([DKV + 2, SQL], bf16, tag="oT")
            recip_sb = ppool.tile([128, SQL // 128], f32, tag="recip")
            wfc_b = wpool.tile([DKV, D], bf16, tag="wfc_b")

            nc.gpsimd.memset(vp_aug[:, :, DKV:DKV + 1], 1.0)

            with tc.tile_pool(name="pa_kv", bufs=1) as pkv_pool, \
                 tc.tile_pool(name="pa_q", bufs=1) as pq_pool, \
                 tc.tile_pool(name="pb_gb", bufs=3) as pgb, \
                 tc.tile_pool(name="pb_m", bufs=3) as pm, \
                 tc.tile_pool(name="pb_e", bufs=16) as pe_pool, \
                 tc.tile_pool(name="pb_o", bufs=2) as po_pool, \
                 tc.tile_pool(name="ps_s", bufs=2, space="PSUM") as ps_s, \
                 tc.tile_pool(name="ps_pv", bufs=1, space="PSUM") as ps_pv, \
                 tc.tile_pool(name="ps_fc", bufs=2, space="PSUM") as ps_fc:

                # ---- plain loads of host-transposed q/k/v ----
                # (DMA-transpose is unusable here: it serializes against all
                # concurrent DMA traffic, and cross-ring transposes race.)
                kT_h0 = pkv_pool.tile([128, 4, SQL], bf16, tag="kT0")
                kT_h1 = pkv_pool.tile([128, 4, SQL], bf16, tag="kT1")
                kT = [kT_h0, kT_h1]
                nc.sync.dma_start(
                    kT[0][:], k_ext[:, 0:SQL].rearrange("(c p) r -> p c r", p=128))
                qT = pq_pool.tile([128, 4, SQL], bf16, tag="qT")
                nc.scalar.dma_start(
                    qT[:], q_ext.rearrange("(c p) r -> p c r", p=128))

                # gb/mask DMAs: mask arrives bf16 {0,1} from the host so
                # both are plain fast HWDGE loads (SWDGE cast was 20us late,
                # DVE casting was 4us/group).
                gb_tiles, m_tiles = {}, {}

                def issue_group_dma(g, gb_eng, m_eng):
                    rows = slice(512 * g, 512 * (g + 1))
                    m_t = pm.tile([128, 4, SQL], bf16, tag="m")
                    m_eng.dma_start(
                        m_t[:], mt_ext[rows].rearrange("(t p) s -> p t s", p=128))
                    gb_t = pgb.tile([128, 4, SQL], bf16, tag="gb")
                    gb_eng.dma_start(
                        gb_t[:], gbt_ext[rows].rearrange("(t p) s -> p t s", p=128))
                    gb_tiles[g], m_tiles[g] = gb_t, m_t

                issue_group_dma(0, nc.sync, nc.sync)
                nc.sync.dma_start(
                    kT[1][:], k_ext[:, SQL:S].rearrange("(c p) r -> p c r", p=128))
                nc.gpsimd.dma_start(wfc_b[:], wfc_ext[:])

                vT = pkv_pool.tile([128, 4, S], bf16, tag="vT")
                nc.scalar.dma_start(
                    vT[:], v_ext.rearrange("(c p) r -> p c r", p=128))
                issue_group_dma(1, nc.scalar, nc.scalar)

                # ---- warm-up + k/q projections (PSUM shared with phase B) ----
                if True:
                    # HAM warm-up: ~4us of junk matmuls while input DMAs fly,
                    # so projections + phase B run at 2.4GHz from the start
                    ps_w = ps_fc.tile([128, 512], f32, tag="fc")
                    for i in range(32):
                        nc.tensor.matmul(ps_w[:], warm_t[:, 0:128], warm_t[:],
                                         start=True, stop=True)
                    # kpT half 0 first: unblocks score tiles 0..7
                    def kproj(half):
                        hsl = slice(SQL * half, SQL * (half + 1))
                        pp = ps_s.tile([DKV, SQL], f32, tag="s")
                        for c in range(2):
                            sl = slice(512 * c, 512 * (c + 1))
                            for j in range(4):
                                nc.tensor.matmul(pp[:, sl], wk_t[:, j, :],
                                                 kT[half][:, j, sl],
                                                 start=(j == 0), stop=(j == 3))
                        nc.vector.tensor_scalar(out=kpT[:, hsl], in0=pp[:],
                                                scalar1=bk_t, scalar2=None,
                                                op0=OP.add)

                    kproj(0)

                    pp = ps_s.tile([DKV, SQL], f32, tag="s")
                    for c in range(2):
                        sl = slice(512 * c, 512 * (c + 1))
                        for j in range(4):
                            nc.tensor.matmul(pp[:, sl], wq_t[:, j, :], qT[:, j, sl],
                                             start=(j == 0), stop=(j == 3))
                    nc.vector.tensor_scalar(out=qpT[:], in0=pp[:], scalar1=bq_t,
                                            scalar2=qs_t, op0=OP.add, op1=OP.mult)

                # ---- phase B: scores, exp, per-tile V projection, PV ----
                if True:
                    oT_ps = ps_pv.tile([DKV + 1, SQL], f32, tag="oTp")
                    e_tiles = {}

                    def pv_mm(j):
                        # PV for tile j, issued one iteration late so the PE
                        # never stalls in-FIFO waiting on ACT's exp.
                        for c in range(2):
                            sl = slice(512 * c, 512 * (c + 1))
                            nc.tensor.matmul(oT_ps[:, sl], vp_aug[:, j, :],
                                             e_tiles[j][:, sl], start=(j == 0),
                                             stop=(j == NT_SK - 1))

                    def vp_proj(j):
                        pv = ps_fc.tile([128, DKV], f32, tag="fc")
                        for vj in range(4):
                            nc.tensor.matmul(pv[:],
                                             vT[:, vj, 128 * j:128 * (j + 1)],
                                             wv_t[:, vj, :], start=(vj == 0),
                                             stop=(vj == 3))
                        nc.vector.tensor_tensor(out=vp_aug[:, j, 0:DKV],
                                                in0=pv[:], in1=bv_t, op=OP.add)

                    for g in range(NG_GB):  # noqa
                        for t in range(4):
                            j = 4 * g + t
                            gb_t, m_t = gb_tiles[g], m_tiles[g]
                            ps = ps_s.tile([128, SQL], f32, tag="s")
                            for c in range(2):
                                sl = slice(512 * c, 512 * (c + 1))
                                nc.tensor.matmul(ps[:, sl],
                                                 kpT[:, 128 * j:128 * (j + 1)],
                                                 qpT[:, sl], start=True, stop=False)
                            for c in range(2):
                                sl = slice(512 * c, 512 * (c + 1))
                                nc.tensor.matmul(ps[:, sl], ident_bf[:],
                                                 gb_t[:, t, sl],
                                                 start=False, stop=False)
                            for c in range(2):
                                sl = slice(512 * c, 512 * (c + 1))
                                nc.tensor.matmul(ps[:, sl], ident_m[:],
                                                 m_t[:, t, sl],
                                                 start=False, stop=True)
                            vp_proj(j)
                            e_t = pe_pool.tile([128, SQL], f16, tag="e")
                            nc.scalar.activation(e_t[:], ps[:], AF.Exp,
                                                 bias=eb_t[:], scale=es_t)
                            e_tiles[j] = e_t
                            if j > 0:
                                pv_mm(j - 1)
                            # stage upcoming groups (issued inside the loop
                            # so tile0's fused PE wait can't include them)
                            if g == 0 and t == 2:
                                kproj(1)
                            if t == 0 and g + 2 < NG_GB:
                                issue_group_dma(g + 2, nc.scalar, nc.scalar)
                    pv_mm(NT_SK - 1)

                    # ---- tail: denominators, FC, store ----
                    nc.scalar.copy(oT_sb[0:DKV + 1, :], oT_ps[:])
                    for t in range(SQL // 128):
                        csl = slice(128 * t, 128 * (t + 1))
                        tr = ps_fc.tile([128, DKV + 2], bf16, tag="fc")
                        nc.tensor.transpose(tr[:], oT_sb[:, csl],
                                            ident_bf[0:DKV + 2, 0:DKV + 2])
                        nc.vector.reciprocal(recip_sb[:, t:t + 1],
                                             tr[:, DKV:DKV + 1])
                    for t in range(SQL // 128):
                        csl = slice(128 * t, 128 * (t + 1))
                        pf = ps_s.tile([128, D], f32, tag="s")
                        nc.tensor.matmul(pf[:], oT_sb[0:DKV, csl], wfc_b[:],
                                         start=True, stop=True)
                        o_sb = po_pool.tile([128, D], f32, tag="o")
                        nc.vector.scalar_tensor_tensor(
                            out=o_sb[:], in0=pf[:], scalar=recip_sb[:, t:t + 1],
                            in1=bfc_t, op0=OP.mult, op1=OP.add)
                        nc.sync.dma_start(out_ext[csl, :], o_sb[:])

    nc.finalize()
    return nc


_cache = {}


def _bf16(x):
    import ml_dtypes
    return np.ascontiguousarray(np.asarray(x, np.float32).astype(ml_dtypes.bfloat16))


def kernel(**inputs):
    from concourse.bass_utils import run_bass_kernel_spmd

    q = np.asarray(inputs["q"], np.float32)
    k = np.asarray(inputs["k"], np.float32)
    v = np.asarray(inputs["v"], np.float32)
    gb = np.asarray(inputs["g_bias"], np.float32)
    mask = np.asarray(inputs["mask"]).astype(np.uint8)
    tau = float(np.asarray(inputs["tau"]))

    if "nc" not in _cache:
        _cache["nc"] = _build()
    nc = _cache["nc"]

    in_maps = build_in_maps(inputs, q, k, v, gb, mask, tau)
    res = run_bass_kernel_spmd(nc, in_maps, list(range(N_CORES)))
    out = np.empty((B, S, D), np.float32)
    for c in range(N_CORES):
        b, h = divmod(c, 2)
        out[b, h * SQL:(h + 1) * SQL] = res.results[c]["out"]
    return out


def build_in_maps(inputs, q, k, v, gb, mask, tau):
    consts = np.zeros((128, 4 + DKV + D), np.float32)
    consts[0:DKV, 0] = np.asarray(inputs["bq"], np.float32)
    consts[0:DKV, 1] = np.asarray(inputs["bk"], np.float32)
    consts[0:DKV, 2] = (2.0 * tau * tau) / 8.0
    consts[:, 3] = 1.0 / (2.0 * tau * tau)
    consts[:, 4:4 + DKV] = np.asarray(inputs["bv"], np.float32)
    consts[:, 4 + DKV:] = np.asarray(inputs["bfc"], np.float32)
    w3 = np.concatenate([np.asarray(inputs["Wq"], np.float32),
                         np.asarray(inputs["Wk"], np.float32),
                         np.asarray(inputs["Wv"], np.float32)], axis=1)
    shared = {
        "w3": _bf16(w3),
        "Wfc": _bf16(inputs["Wfc"]),
        "consts": consts,
    }
    kb = [_bf16(k[b].T) for b in range(B)]
    vb = [_bf16(v[b].T) for b in range(B)]
    in_maps = []
    for c in range(N_CORES):
        b, h = divmod(c, 2)
        sl = slice(h * SQL, (h + 1) * SQL)
        in_maps.append({
            "q": _bf16(q[b, sl].T),
            "k": kb[b],
            "v": vb[b],
            "gbt": _bf16(gb[b, sl].T),
            "maskt": _bf16(mask[b, sl].T),
            **shared,
        })
    return in_maps
